# revision 1
# baseline (speedup 1.0000x reference)
"""GRU predictor kernel for 8 TRN2 NeuronCores (data-parallel over batch).

Reference semantics (PyTorch GRU gate order r, z, n):
    gx = x @ w_ih.T + b_ih            # per step: [B, 3H]
    gh = h @ w_hh.T + b_hh
    r = sigmoid(gx_r + gh_r)
    z = sigmoid(gx_z + gh_z)
    n = tanh(gx_n + r * gh_n)         # gh_n includes b_hh_n
    h = (1 - z) * n + z * h
    out = h_T @ fc_w.T + fc_b

Shapes: B=512, T=2048, I=8, H=128, O=96. Sharding: batch/8 -> 64 per core.

On-chip layout: partition dim = H (128), free dim = local batch (64).
h lives as hT [H, B]; all gate math in that layout; x is pre-transposed
on host to xq [I=8, T*B] so each step's matmul rhs is a free-dim slice.

The recurrence is latency-bound: per step the dependency chain is
PE(matmul) -> ACT(sigmoid r) -> DVE(r*ghn + gxn) -> ACT(tanh) -> DVE(mix) -> PE.
"""

import numpy as np

B, T, I, H, O = 512, 2048, 8, 128, 96
NCORES = 8
BL = B // NCORES  # 64 local batch
CHUNK = 128       # timesteps per x DMA chunk
NCHUNK = T // CHUNK  # recomputed inside _build_nc for custom T


def _build_nc(T=T, T_dram=None):
    import concourse.bass as bass
    import concourse.mybir as mybir

    f32 = mybir.dt.float32
    AF = mybir.ActivationFunctionType
    ALU = mybir.AluOpType

    T_dram = T_dram or T
    nchunk = T // CHUNK
    nc = bass.Bass()

    xq = nc.dram_tensor("xq", [I, T_dram * BL], f32, kind="ExternalInput")
    whh = nc.dram_tensor("whh", [H, 3 * H], f32, kind="ExternalInput")
    wih = nc.dram_tensor("wih", [I, 3 * H], f32, kind="ExternalInput")
    bias = nc.dram_tensor("bias", [H, 6], f32, kind="ExternalInput")
    fcw = nc.dram_tensor("fcw", [H, O], f32, kind="ExternalInput")
    out = nc.dram_tensor("out", [O, BL], f32, kind="ExternalOutput")

    from contextlib import ExitStack

    with ExitStack() as st:
        e = st.enter_context
        whh_sb = e(nc.sbuf_tensor([H, 3 * H], f32))
        wih_sb = e(nc.sbuf_tensor([I, 3 * H], f32))
        bias_sb = e(nc.sbuf_tensor([H, 6], f32))
        fcw_sb = e(nc.sbuf_tensor([H, O], f32))
        xc0 = e(nc.sbuf_tensor([I, CHUNK * BL], f32))
        xc1 = e(nc.sbuf_tensor([I, CHUNK * BL], f32))
        h0_sb = e(nc.sbuf_tensor([H, BL], f32))
        h1_sb = e(nc.sbuf_tensor([H, BL], f32))
        r_sb = e(nc.sbuf_tensor([H, BL], f32))
        z_sb = e(nc.sbuf_tensor([H, BL], f32))
        zp_sb = e(nc.sbuf_tensor([H, BL], f32))
        n_sb = e(nc.sbuf_tensor([H, BL], f32))
        t3_sb = e(nc.sbuf_tensor([H, BL], f32))
        u2_sb = e(nc.sbuf_tensor([H, BL], f32))
        v1_sb = e(nc.sbuf_tensor([H, BL], f32))
        o_sb = e(nc.sbuf_tensor([O, BL], f32))
        ps_r0 = e(nc.psum_tensor([H, BL], f32))
        ps_r1 = e(nc.psum_tensor([H, BL], f32))
        ps_z0 = e(nc.psum_tensor([H, BL], f32))
        ps_z1 = e(nc.psum_tensor([H, BL], f32))
        ps_nh0 = e(nc.psum_tensor([H, BL], f32))
        ps_nh1 = e(nc.psum_tensor([H, BL], f32))
        ps_nx0 = e(nc.psum_tensor([H, BL], f32))
        ps_nx1 = e(nc.psum_tensor([H, BL], f32))
        sem_ld = e(nc.semaphore())
        sem_x = e(nc.semaphore())
        sem_pe = e(nc.semaphore())
        sem_act = e(nc.semaphore())
        sem_dve = e(nc.semaphore())
        sem_h = e(nc.semaphore())
        sem_out = e(nc.semaphore())
        block = e(nc.Block())
        xc = [xc0, xc1]
        h_sb = [h0_sb, h1_sb]
        ps_r = [ps_r0, ps_r1]
        ps_z = [ps_z0, ps_z1]
        ps_nh = [ps_nh0, ps_nh1]
        ps_nx = [ps_nx0, ps_nx1]

        # bias columns: 0=r (bih+bhh), 1=z (bih+bhh), 2=-z, 3=nh (bhh_n), 4=nx (bih_n), 5=fc (pad to 128)
        b_r = bias_sb[:, 0:1]
        b_z = bias_sb[:, 1:2]
        b_nz = bias_sb[:, 2:3]
        b_nh = bias_sb[:, 3:4]
        b_nx = bias_sb[:, 4:5]
        b_fc = bias_sb[0:O, 5:6]

        @block.sync
        def _(sync):
            sync.dma_start(out=whh_sb[:], in_=whh[:]).then_inc(sem_ld, 16)
            sync.dma_start(out=wih_sb[:], in_=wih[:]).then_inc(sem_ld, 16)
            sync.dma_start(out=bias_sb[:], in_=bias[:]).then_inc(sem_ld, 16)
            sync.dma_start(out=fcw_sb[:], in_=fcw[:]).then_inc(sem_ld, 16)
            for c in range(nchunk):
                if c >= 2:
                    # buffer c%2 is free once PE consumed chunk c-2
                    sync.wait_ge(sem_pe, 4 * CHUNK * (c - 1))
                sync.dma_start(
                    out=xc[c % 2][:],
                    in_=xq[:, c * CHUNK * BL:(c + 1) * CHUNK * BL],
                ).then_inc(sem_x, 16)
            sync.wait_ge(sem_out, 1)
            sync.dma_start(out=out[:], in_=o_sb[:]).then_inc(sem_x, 16)

        @block.tensor
        def _(pe):
            for t in range(T):
                s = t % 2
                c, j = divmod(t, CHUNK)
                xsl = xc[c % 2][:, j * BL:(j + 1) * BL]
                # x-side matmuls carry no h dependency: issue ahead of the
                # sem_h stall so only one matmul (mm_hr) sits on the chain.
                mm_xn = pe.matmul(ps_nx[s][:], wih_sb[:, 2 * H:3 * H], xsl,
                                  start=True, stop=True)
                if j == 0:
                    mm_xn._wait_ge(sem_x, 16 * (c + 1))
                mm_xn.then_inc(sem_pe, 1)
                pe.matmul(ps_r[s][:], wih_sb[:, 0:H], xsl,
                          start=True, stop=False)
                mm_hr = pe.matmul(ps_r[s][:], whh_sb[:, 0:H], h_sb[s][:],
                                  start=False, stop=True)
                mm_hr._wait_ge(sem_h, t + 1)
                mm_hr.then_inc(sem_pe, 1)
                pe.matmul(ps_nh[s][:], whh_sb[:, 2 * H:3 * H], h_sb[s][:],
                          start=True, stop=True).then_inc(sem_pe, 1)
                pe.matmul(ps_z[s][:], wih_sb[:, H:2 * H], xsl,
                          start=True, stop=False)
                pe.matmul(ps_z[s][:], whh_sb[:, H:2 * H], h_sb[s][:],
                          start=False, stop=True).then_inc(sem_pe, 1)
            mmo = pe.matmul(ps_r[0][0:O, :], fcw_sb[:], h_sb[0][:],
                            start=True, stop=True)
            mmo._wait_ge(sem_h, T + 1)
            mmo.then_inc(sem_pe, 1)

        @block.scalar
        def _(act):
            for t in range(T):
                s = t % 2
                a_r = act.activation(r_sb[:], ps_r[s][:], AF.Sigmoid, bias=b_r)
                a_r._wait_ge(sem_pe, 4 * t + 2)
                a_r.then_inc(sem_act, 1)
                a_z = act.activation(z_sb[:], ps_z[s][:], AF.Sigmoid, bias=b_z)
                a_z._wait_ge(sem_pe, 4 * t + 4)
                a_z.then_inc(sem_act, 1)
                a_n = act.activation(n_sb[:], ps_nx[s][:], AF.Tanh, bias=b_nx)
                a_n._wait_ge(sem_dve, t + 1)
                a_n.then_inc(sem_act, 1)
            a_o = act.activation(o_sb[:], ps_r[0][0:O, :], AF.Identity, bias=b_fc)
            a_o._wait_ge(sem_pe, 4 * T + 1)
            a_o.then_inc(sem_out, 1)

        @block.vector
        def _(dve):
            i_ms = dve.memset(h_sb[0][:], 0.0)
            i_ms._wait_ge(sem_ld, 64)
            i_ms.then_inc(sem_h, 1)
            for t in range(T):
                s = t % 2
                # t3 = (ps_nh + b_nh) * r   [ps_nh ready: mm_hn retires
                # ~107ns after mm_hr; r's ACT+2 sem hops take >400ns]
                i_t3 = dve.scalar_tensor_tensor(
                    t3_sb[:], ps_nh[s][:], b_nh, r_sb[:], ALU.add, ALU.mult)
                i_t3._wait_ge(sem_act, 3 * t + 1)
                # ps_nx <- t3 + ps_nx   (tanh arg; b_nx added by ACT tanh)
                dve.tensor_tensor(ps_nx[s][:], t3_sb[:], ps_nx[s][:],
                                  ALU.add).then_inc(sem_dve, 1)
                # zp = 1 - z  (off the ACT queue so tanh isn't FIFO-delayed)
                i_zp = dve.tensor_scalar(zp_sb[:], z_sb[:], -1.0, 1.0,
                                         ALU.mult, ALU.add)
                i_zp._wait_ge(sem_act, 3 * t + 2)
                # u2 = z * h
                dve.tensor_tensor(u2_sb[:], z_sb[:], h_sb[s][:], ALU.mult)
                # v1 = (1-z) * n
                i_v1 = dve.tensor_tensor(v1_sb[:], zp_sb[:], n_sb[:], ALU.mult)
                i_v1._wait_ge(sem_act, 3 * t + 3)
                # h' = v1 + u2
                dve.tensor_tensor(h_sb[1 - s][:], v1_sb[:], u2_sb[:],
                                  ALU.add).then_inc(sem_h, 1)

    return nc


_NC_CACHE = {}


def _get_nc():
    if "nc" not in _NC_CACHE:
        _NC_CACHE["nc"] = _build_nc()
    return _NC_CACHE["nc"]


def kernel(x, w_ih, w_hh, b_ih, b_hh, fc_w, fc_b):
    from concourse.bass_utils import run_bass_kernel_spmd

    x = np.asarray(x, dtype=np.float32)
    w_ih = np.asarray(w_ih, dtype=np.float32)
    w_hh = np.asarray(w_hh, dtype=np.float32)
    b_ih = np.asarray(b_ih, dtype=np.float32)
    b_hh = np.asarray(b_hh, dtype=np.float32)
    fc_w = np.asarray(fc_w, dtype=np.float32)
    fc_b = np.asarray(fc_b, dtype=np.float32)

    whh_np = np.ascontiguousarray(w_hh.T)               # [H, 3H]
    wih_np = np.ascontiguousarray(w_ih.T)               # [I, 3H]
    fcw_np = np.ascontiguousarray(fc_w.T)               # [H, O]
    bias_np = np.zeros((H, 6), dtype=np.float32)
    bias_np[:, 0] = b_ih[0:H] + b_hh[0:H]
    bias_np[:, 1] = b_ih[H:2 * H] + b_hh[H:2 * H]
    bias_np[:, 2] = -bias_np[:, 1]
    bias_np[:, 3] = b_hh[2 * H:3 * H]
    bias_np[:, 4] = b_ih[2 * H:3 * H]
    bias_np[0:O, 5] = fc_b

    in_maps = []
    for k in range(NCORES):
        xk = x[k * BL:(k + 1) * BL]                     # [BL, T, I]
        xqk = np.ascontiguousarray(
            xk.transpose(2, 1, 0).reshape(I, T * BL))   # [I, (t, b)]
        in_maps.append({
            "xq": xqk,
            "whh": whh_np,
            "wih": wih_np,
            "bias": bias_np,
            "fcw": fcw_np,
        })

    nc = _get_nc()
    res = run_bass_kernel_spmd(nc, in_maps, list(range(NCORES)))

    out = np.empty((B, O), dtype=np.float32)
    for k in range(NCORES):
        out[k * BL:(k + 1) * BL] = res.results[k]["out"].T
    return out

